# revision 96
# baseline (speedup 1.0000x reference)
"""Gemma sliding-window attention layer on 8 Trainium2 NeuronCores.

Sharding: data-parallel over batch (B=2) x tensor-parallel over heads
(4 groups: 2 q heads + 1 kv head each) = 8 cores. Each core computes a
partial o-proj output [D, S] in bf16; host sums the 4 TP partials per
batch in fp32 and transposes back to [S, D].

Matmul precision: projections, o-proj AND attention scores run as fp8e4
DoubleRow with a hi/lo residual split (x ~= hi + lo, both e4m3); the
3-term product Wh@xh + Wh@xl + Wl@xh carries ~0.1-0.2% relative error
at 0.75x the bf16 matmul cost. A@V stays bf16 (the elementwise cost of
splitting exp() into hi/lo exceeds the PE savings). Softmax
denominators sum a plain fp8 cast of exp() via a 1/32-ones DoubleRow
matmul (quantization washes out over the window; the 1/32 folds S_AO
so tt = 32*attn lands pre-scaled for the o-proj's fp8 ao split).

Scheduling (cross-phase software pipeline): attention scores run two
(sub,head) groups ahead of A@V/dn to cover the exp->mask->cast chain;
the next block's k projection row and the previous block's deferred
sub-1 o-proj groups run as PE fillers inside the attention phase; the
next block's q rows interleave with this block's o-proj; rms sums and
softmax denominators use all-ones stationary matmuls so results land
pre-broadcast across psum partitions; rope reads go through one
psum->sbuf bf16 copy per half so rotation runs in DVE 2x mode;
latency-critical elementwise (k-row casts, exp fp8 casts, masks) sits
on the DVE, slack work (q-row lo/hi, ao lo) on the Pool.

Layouts on device (per core):
  qhi/qlo (per head), khi/klo: [128, 2, S] fp8 (dh-half pairs on the
  DoubleRow axis) after rmsnorm+rope; q carries x16 scale, exp applies
  1/256 + a -ln4 bias fold.
  v: [S(chunked), DH] bf16; exp tiles [k=128, 2-chunk pair, q=256];
  scores^T tiles so no transposes are needed anywhere.
"""

import sys

sys.path.insert(0, "/opt/trn_rl_repo")

from contextlib import ExitStack

import numpy as np
import ml_dtypes

import concourse.bass as bass
import concourse.tile as tile
from concourse import bacc, mybir
from concourse.bass import ds, ts
from concourse.bass_utils import run_bass_kernel_spmd

BF16 = mybir.dt.bfloat16
F32 = mybir.dt.float32
FP8 = mybir.dt.float8e4
NPBF16 = ml_dtypes.bfloat16
NPFP8 = ml_dtypes.float8_e4m3fn
DRM = mybir.MatmulPerfMode.DoubleRow
AF = mybir.ActivationFunctionType

H, KVH, DH, SW = 8, 4, 256, 1024
B, S, D = 2, 2048, 2048
EPS = 1e-6
ROPE_BASE = 10000.0
P = 128
SB = 512          # s-block width
NSB = S // SB     # 4
NDC = D // P      # 16 contraction chunks
EQ = 2 * DH       # per-core q width (2 heads)

ATT_QB = 256      # attention q-tile width

SW_W = 2048.0     # weight fp8 scale
SW_X = 32.0       # hidden-state fp8 scale
CSC = SW_W * SW_X          # combined matmul scale (2^16)
C2 = CSC * CSC             # 2^32
S_AO = 32.0                # attention-output fp8 scale
SQ16 = 16.0                # q fp8 scale (k carries 1/16 fold -> net 1/256 at exp)
S_V = 32.0                 # v fp8 scale
S_P = 4.0                  # exp() fp8 scale
NEGLNP = -1.3862943611198906  # -ln(S_P)


def _kpairs(q0, qb):
    """k-chunk pairs (kc0, masktype) for q-tile [q0, q0+qb).

    Chunks come in DoubleRow pairs (kc0, kc0+1). masktype: 0 = causal
    pair (d=0,128: combined mask row 0), 1 = window pair (d=-SW,
    -SW+128: combined mask row 1), None = fully valid pair.
    """
    KCs = list(range(max(0, q0 - SW), q0 + qb, P))
    assert len(KCs) % 2 == 0
    res = []
    for a in range(0, len(KCs), 2):
        d = KCs[a] - q0
        assert KCs[a + 1] == KCs[a] + P
        mt = 0 if d == 0 else (1 if d + SW == 0 else None)
        res.append((KCs[a] // P, mt))
    return res


def _build():
    nc = bacc.Bacc("TRN2", target_bir_lowering=False, debug=False)

    # host-prearranged tensors: sbuf layout already, identity DMA copies
    xth_d = nc.dram_tensor("xth", [P, NSB, NDC, SB], FP8, kind="ExternalInput")
    xtl_d = nc.dram_tensor("xtl", [P, NSB, NDC, SB], FP8, kind="ExternalInput")
    # head-major so each q head's weights are one contiguous 4KB-run DMA
    wqh_d = nc.dram_tensor("wqh", [P, 2, NDC * DH], FP8, kind="ExternalInput")
    wql_d = nc.dram_tensor("wql", [P, 2, NDC * DH], FP8, kind="ExternalInput")
    # flat layout: 4KB innermost contiguous runs (<512B runs pay a 2x DMA
    # latency multiplier, and DH=256B would)
    wkh_d = nc.dram_tensor("wkh", [P, NDC * DH], FP8, kind="ExternalInput")
    wkl_d = nc.dram_tensor("wkl", [P, NDC * DH], FP8, kind="ExternalInput")
    wvh_d = nc.dram_tensor("wvh", [P, NDC * DH], FP8, kind="ExternalInput")
    wvl_d = nc.dram_tensor("wvl", [P, NDC * DH], FP8, kind="ExternalInput")
    woh_d = nc.dram_tensor("woh", [P, 4, D], FP8, kind="ExternalInput")
    wol_d = nc.dram_tensor("wol", [P, 4, D], FP8, kind="ExternalInput")
    trig = nc.dram_tensor("trig", [2, P, S], BF16, kind="ExternalInput")  # cos, sin
    masks = nc.dram_tensor("masks", [2, P, 2 * ATT_QB], BF16, kind="ExternalInput")
    o128f8 = nc.dram_tensor("o128f8", [P, 2, P], FP8, kind="ExternalInput")  # 1.0
    o32f8 = nc.dram_tensor("o32f8", [P, 2, P], FP8, kind="ExternalInput")  # 1/32
    out = nc.dram_tensor("out", [D, S], BF16, kind="ExternalOutput")

    cbias = nc.dram_tensor("cbias", [P, 4], F32, kind="ExternalInput")

    with tile.TileContext(nc) as tc, ExitStack() as ctx:
        sp = ctx.enter_context(tc.tile_pool(name="sp", bufs=2))    # SBUF
        pp = ctx.enter_context(tc.tile_pool(name="pp", bufs=2, space="PSUM"))

        # ---- persistent SBUF tiles ----
        wqh_sb = sp.tile([P, 2, NDC, DH], FP8, name="wqh_sb", tag="wqh", bufs=1)
        wql_sb = sp.tile([P, 2, NDC, DH], FP8, name="wql_sb", tag="wql", bufs=1)
        wkh_sb = sp.tile([P, NDC, DH], FP8, name="wkh_sb", tag="wkh", bufs=1)
        wkl_sb = sp.tile([P, NDC, DH], FP8, name="wkl_sb", tag="wkl", bufs=1)
        wvh_sb = sp.tile([P, NDC, DH], FP8, name="wvh_sb", tag="wvh", bufs=1)
        wvl_sb = sp.tile([P, NDC, DH], FP8, name="wvl_sb", tag="wvl", bufs=1)
        woh_sb = sp.tile([P, 4, D], FP8, name="woh_sb", tag="woh", bufs=1)
        wol_sb = sp.tile([P, 4, D], FP8, name="wol_sb", tag="wol", bufs=1)
        msk_sb = sp.tile([P, 2, 2 * ATT_QB], BF16, name="msk", tag="msk", bufs=1)
        o128_sb = sp.tile([P, 2, P], FP8, name="o128_sb", tag="o128", bufs=1)
        o32_sb = sp.tile([P, 2, P], FP8, name="o32_sb", tag="o32", bufs=1)
        # q/k fp8 hi/lo: [P, 2(dh-half), S]
        q_hi = [sp.tile([P, 2, S], FP8, name=f"qhi{i}", tag="qsb", bufs=4) for i in range(2)]
        q_lo = [sp.tile([P, 2, S], FP8, name=f"qlo{i}", tag="qsb", bufs=4) for i in range(2)]
        k_hi = sp.tile([P, 2, S], FP8, name="khi", tag="ksb", bufs=2)
        k_lo = sp.tile([P, 2, S], FP8, name="klo", tag="ksb", bufs=2)
        v_sb = sp.tile([P, NDC, DH], BF16, name="vsb", tag="vsb", bufs=1)
        aoh_sb = sp.tile([P, 4, S], FP8, name="aoh_sb", tag="aoh", bufs=1)
        aol_sb = sp.tile([P, 4, S], FP8, name="aol_sb", tag="aol", bufs=1)

        xt_tiles = {}

        def xt_piece(t, dram, blk, a, n):
            nc.sync.dma_start(
                t[:, ds(a, n), :],
                dram[:, ds(blk, 1), ds(a, n), :].rearrange("p b c s -> p (b c) s"))

        def trig_dma(tgt, blk):
            nc.sync.dma_start(tgt[:].rearrange("p (r s) -> p r s", r=2),
                              trig.rearrange("r p s -> p r s")[:, :, ds(blk * SB, SB)])

        def issue_xt(blk):
            xth_t = sp.tile([P, NDC, SB], FP8, name=f"xth{blk}", tag="xth", bufs=2)
            xtl_t = sp.tile([P, NDC, SB], FP8, name=f"xtl{blk}", tag="xtl", bufs=2)
            tgt = sp.tile([P, 2 * SB], BF16, name=f"tg{blk}", tag="tg", bufs=2)
            xt_piece(xth_t, xth_d, blk, 0, NDC)
            xt_piece(xtl_t, xtl_d, blk, 0, NDC)
            trig_dma(tgt, blk)
            xt_tiles[blk] = (xth_t, xtl_t, tgt)

        # startup DMAs ordered by first use: tiny starter pieces for the k
        # hh pass (first 2 contraction chunks of wkh + xth) so the PE can
        # begin ~1us in; the rest streams behind in first-use order.
        xth0 = sp.tile([P, NDC, SB], FP8, name="xth0", tag="xth", bufs=2)
        xtl0 = sp.tile([P, NDC, SB], FP8, name="xtl0", tag="xtl", bufs=2)
        tg0 = sp.tile([P, 2 * SB], BF16, name="tg0", tag="tg", bufs=2)
        # activation bias constants arrive by DMA (no memset+barrier
        # preamble); registered now, loaded after the first weight pieces
        cb_sb = sp.tile([P, 4], F32, name="cb_sb", tag="cb", bufs=1)
        for i, val in enumerate((0.0, C2 * EPS, C2 * EPS / 256.0, NEGLNP)):
            nc.const_aps.aps[(F32, val)] = cb_sb[:, ds(i, 1)]
        wkh_f = wkh_sb[:].rearrange("p c d -> p (c d)")
        wkl_f = wkl_sb[:].rearrange("p c d -> p (c d)")
        def wq_dma(dst, src, h, eng=None):
            (eng or nc.sync).dma_start(
                dst[:, ds(h, 1), :, :].rearrange("p a c d -> p (a c d)"),
                src[:, ds(h, 1), :].rearrange("p a f -> p (a f)"))

        nc.sync.dma_start(wkh_f[:, ds(0, 2 * DH)], wkh_d[:, ds(0, 2 * DH)])
        xt_piece(xth0, xth_d, 0, 0, 2)
        nc.sync.dma_start(wkh_f[:, ds(2 * DH, 14 * DH)], wkh_d[:, ds(2 * DH, 14 * DH)])
        xt_piece(xth0, xth_d, 0, 2, 6)
        wq_dma(wqh_sb, wqh_d, 0)
        xt_piece(xth0, xth_d, 0, 8, 8)
        nc.sync.dma_start(wkl_f, wkl_d[:])
        wq_dma(wql_sb, wql_d, 0)
        nc.sync.dma_start(cb_sb[:], cbias[:])
        nc.sync.dma_start(o128_sb[:], o128f8[:])
        nc.sync.dma_start(o32_sb[:], o32f8[:])
        xt_piece(xtl0, xtl_d, 0, 0, 8)
        xt_piece(xtl0, xtl_d, 0, 8, 8)
        wq_dma(wqh_sb, wqh_d, 1)
        trig_dma(tg0, 0)
        wq_dma(wql_sb, wql_d, 1)
        nc.sync.dma_start(wvh_sb[:].rearrange("p c d -> p (c d)"), wvh_d[:])
        nc.sync.dma_start(wvl_sb[:].rearrange("p c d -> p (c d)"), wvl_d[:])
        xt_tiles[0] = (xth0, xtl0, tg0)

        out_r = out.rearrange("(g p) s -> p g s", p=P)
        pending_oproj = []

        def emit_oproj(blk, sub, g4, split_dma=False, dve_copies=False):
            qsl = ds(blk * SB + sub * ATT_QB, ATT_QB)
            ob4 = sp.tile([P, 4 * ATT_QB], BF16, name=f"ob_{blk}_{g4}_{sub}",
                          tag="ob", bufs=8)
            for j2 in range(2):
                # dmc pair shares one psum bank; one copy per pair
                op = pp.tile([P, 2 * ATT_QB], F32,
                             name=f"op_{blk}_{g4}_{j2}_{sub}", tag="mm", bufs=4)
                for ji in range(2):
                    dmc = 4 * g4 + 2 * j2 + ji
                    i = 0
                    for wt, at_ in ((woh_sb, aoh_sb), (woh_sb, aol_sb),
                                    (wol_sb, aoh_sb)):
                        for e2 in range(2):
                            nc.tensor.matmul(
                                op[:, ds(ji * ATT_QB, ATT_QB)],
                                wt[:, ds(2 * e2, 2), ds(dmc * P, P)],
                                at_[:, ds(2 * e2, 2), qsl],
                                start=(ji == 0 and i == 0),
                                stop=(ji == 1 and i == 5), perf_mode=DRM)
                            i += 1
                dst = ob4[:, ds(2 * j2 * ATT_QB, 2 * ATT_QB)]
                # alternate copy engines so psum "mm" slots free faster
                if j2 % 2 == 0 and not dve_copies:
                    nc.scalar.activation(dst, op[:], AF.Copy, scale=1.0 / CSC)
                else:
                    nc.vector.tensor_scalar_mul(dst, op[:], 1.0 / CSC)
                if split_dma:
                    # tail: per-pair DMA right after its copy
                    nc.sync.dma_start(
                        out_r[:, ds(4 * g4 + 2 * j2, 2), qsl],
                        dst.rearrange("p (g s) -> p g s", g=2))
            if not split_dma:
                nc.sync.dma_start(
                    out_r[:, ds(4 * g4, 4), qsl],
                    ob4[:].rearrange("p (g s) -> p g s", g=4))

        # ---- attention pipeline stages ----
        def att_scores(blk, sub, h, fill=None):
            """scores + exp + mask + hi/lo split for group (sub, h);
            returns state consumed by att_avdn. `fill` interleaves PE
            filler work between score pairs (psum ring relief)."""
            q0 = blk * SB + sub * ATT_QB
            qslice = ds(q0, ATT_QB)
            # masked pairs first: their elementwise chain has an extra mask
            # stage, and A@V accumulation order is free
            pairs = sorted(_kpairs(q0, ATT_QB),
                           key=lambda pr: 2 if pr[1] is None else pr[1])
            qh, ql = q_hi[h], q_lo[h]
            phl = []
            for pi, (kc0, mt) in enumerate(pairs):
                if fill is not None and pi in (1, 3):
                    fill(1)
                scp = pp.tile([P, 2 * ATT_QB], F32, name=f"sc_{q0}_{h}_{pi}",
                              tag="mm", bufs=4)
                pt2 = sp.tile([P, 2 * ATT_QB], BF16, name=f"pt_{q0}_{h}_{pi}",
                              tag="pt", bufs=12)
                for ci in range(2):
                    # boundary chunks skip their fully-masked q half
                    col, qsl2 = ds(ci * ATT_QB, ATT_QB), qslice
                    if mt == 0 and ci == 1:
                        col, qsl2 = ds(ci * ATT_QB + P, P), ds(q0 + P, P)
                    elif mt == 1 and ci == 0:
                        col, qsl2 = ds(0, P), ds(q0, P)
                    ksl = ts(kc0 + ci, P)
                    i = 0
                    for kt, qt in ((k_hi, qh), (k_hi, ql), (k_lo, qh)):
                        nc.tensor.matmul(scp[:, col], kt[:, :, ksl],
                                         qt[:, :, qsl2],
                                         start=(ci == 0 and i == 0),
                                         stop=(ci == 1 and i == 2),
                                         perf_mode=DRM, skip_group_check=True)
                        i += 1
                # exp skips the never-written psum slice (avoids reading
                # stale psum); a memset zeroes the matching pt2 slice
                if mt == 0:
                    nc.scalar.activation(pt2[:, ds(0, ATT_QB)],
                                         scp[:, ds(0, ATT_QB)], AF.Exp,
                                         bias=NEGLNP, scale=1.0 / 256.0)
                    nc.scalar.activation(pt2[:, ds(ATT_QB + P, P)],
                                         scp[:, ds(ATT_QB + P, P)], AF.Exp,
                                         bias=NEGLNP, scale=1.0 / 256.0)
                    nc.gpsimd.memset(pt2[:, ds(ATT_QB, P)], 0.0)
                elif mt == 1:
                    nc.scalar.activation(pt2[:, ds(0, P)],
                                         scp[:, ds(0, P)], AF.Exp,
                                         bias=NEGLNP, scale=1.0 / 256.0)
                    nc.scalar.activation(pt2[:, ds(ATT_QB, ATT_QB)],
                                         scp[:, ds(ATT_QB, ATT_QB)], AF.Exp,
                                         bias=NEGLNP, scale=1.0 / 256.0)
                    nc.gpsimd.memset(pt2[:, ds(P, P)], 0.0)
                else:
                    nc.scalar.activation(pt2[:], scp[:], AF.Exp,
                                         bias=NEGLNP, scale=1.0 / 256.0)
                if mt is not None:
                    nc.vector.tensor_mul(
                        pt2[:], pt2[:],
                        msk_sb[:, ds(mt, 1), :].rearrange("p a q -> p (a q)"))
                ph = sp.tile([P, 2 * ATT_QB], FP8, name=f"ph_{q0}_{h}_{pi}",
                             tag="ph", bufs=12)
                # fp8 cast feeds only the denominator sum (errors wash out
                # over the window); DVE copies take the fast path (327ns)
                nc.vector.tensor_copy(ph[:], pt2[:])
                phl.append((kc0, mt, ph, pt2))
            return (q0, h, phl)

        def att_avdn(state):
            """A@V + denominators + normalize + ao hi/lo for one group."""
            q0, h, phl = state
            qslice = ds(q0, ATT_QB)
            npair = len(phl)
            # both dh-halves share one psum bank (single-start column groups)
            ao = pp.tile([P, 2 * ATT_QB], F32, name=f"ao_{q0}_{h}", tag="ao", bufs=2)
            dnp = pp.tile([P, ATT_QB], F32, name=f"dn_{q0}_{h}", tag="dn", bufs=1)
            # dn first (needs only the fp8 casts, ready earliest); the 1/32
            # ones fold S_AO so tt lands pre-scaled for the fp8 ao split
            for pi, (kc0, mt, ph, pt2) in enumerate(phl):
                nc.tensor.matmul(dnp[:], o32_sb[:],
                                 ph[:].rearrange("p (t s) -> p t s", t=2),
                                 start=(pi == 0), stop=(pi == npair - 1),
                                 perf_mode=DRM)
            # A@V in bf16 straight off the exp tiles (no lo-split needed:
            # the elementwise cost of fp8 A@V exceeds its PE savings).
            # Boundary chunks skip their fully-masked q half (exact zeros):
            # causal pair's 2nd chunk covers only q[128:), window pair's
            # 1st chunk only q[:128).
            for pi, (kc0, mt, ph, pt2) in enumerate(phl):
                for half in range(2):
                    vsl = ds(half * P, P)
                    for ci in range(2):
                        qoff, qw = 0, ATT_QB
                        if mt == 0 and ci == 1:
                            qoff, qw = P, P
                        elif mt == 1 and ci == 0:
                            qoff, qw = 0, P
                        nc.tensor.matmul(
                            ao[:, ds(half * ATT_QB + qoff, qw)],
                            v_sb[:, ds(kc0 + ci, 1), vsl].rearrange(
                                "p c d -> p (c d)"),
                            pt2[:, ds(ci * ATT_QB + qoff, qw)],
                            start=(pi == 0 and half == 0 and ci == 0),
                            stop=(pi == npair - 1 and half == 1 and ci == 1),
                            skip_group_check=True)
            db = sp.tile([P, ATT_QB], F32, name=f"db_{q0}_{h}", tag="db", bufs=6)
            nc.vector.reciprocal(db[:], dnp[:])
            for half in range(2):
                ec = 2 * h + half
                tt = sp.tile([P, ATT_QB], BF16, name=f"tt_{q0}_{ec}", tag="tdn", bufs=4)
                hiv = aoh_sb[:, ds(ec, 1), qslice].rearrange("p a q -> p (a q)")
                lov = aol_sb[:, ds(ec, 1), qslice].rearrange("p a q -> p (a q)")
                # NB: gpsimd cannot access PSUM (hw constraint) - DVE here
                nc.vector.tensor_mul(tt[:], ao[:, ds(half * ATT_QB, ATT_QB)], db[:])
                nc.scalar.activation(hiv, tt[:], AF.Copy)
                # aol gates only the next block's o-proj: Pool's latency is
                # fine there and it relieves the DVE
                nc.gpsimd.tensor_sub(lov, tt[:], hiv)

        # ---- phase-1 row machinery (callable out of order for software
        # pipelining: the next block's k row runs as PE filler inside the
        # current block's attention phase) ----
        rowdefs = {"k": (wkh_sb, wkl_sb, None), "q0": (wqh_sb, wql_sb, 0),
                   "q1": (wqh_sb, wql_sb, 1)}
        # rr consts: k keeps unit scale (1/16 attn scale nets against the
        # x16 fp8 fold), q folds x16 for fp8
        rowconst = {"k": (C2 / DH, C2 * EPS),
                    "q0": (C2 / DH / 256.0, C2 * EPS / 256.0),
                    "q1": (C2 / DH / 256.0, C2 * EPS / 256.0)}
        rowtiles = {"k": (k_hi, k_lo), "q0": (q_hi[0], q_lo[0]),
                    "q1": (q_hi[1], q_lo[1])}
        braw, bp16, bsums = {}, {}, {}

        def proj_pass(blk, ent, ti):
            """one hi/lo term pass for one 256-wide projection row; the
            last pass also emits sumsq + psum->bf16 copies (frees psum)."""
            xth_t, xtl_t, _ = xt_tiles[blk]
            whi, wlo, head = rowdefs[ent]
            last = ti == 2
            if (blk, ent) not in braw:
                braw[(blk, ent)] = (
                    pp.tile([P, SB], F32, name=f"pa_{blk}_{ent}", tag="mm", bufs=4),
                    pp.tile([P, SB], F32, name=f"pb_{blk}_{ent}", tag="mm", bufs=4))
            pa, pb = braw[(blk, ent)]
            wt, xt_ = (((whi, xth_t), (wlo, xth_t), (whi, xtl_t)))[ti]
            for pt_, eo in ((pa, 0), (pb, P)):
                for d8 in range(8):
                    if head is None:
                        wsl = wt[:, ds(2 * d8, 2), ds(eo, P)]
                    else:
                        wsl = wt[:, ds(head, 1), ds(2 * d8, 2),
                                 ds(eo, P)].rearrange("p a c e -> p (a c) e")
                    nc.tensor.matmul(
                        pt_[:], wsl, xt_[:, ds(2 * d8, 2), :],
                        start=(ti == 0 and d8 == 0),
                        stop=(last and d8 == 7), perf_mode=DRM)
            if last:
                sq = sp.tile([P, 2 * SB], FP8, name=f"sq_{blk}_{ent}",
                             tag="sq", bufs=4)
                nc.scalar.activation(sq[:, ds(0, SB)], pa[:], AF.Square,
                                     bias=0.0, scale=1.0 / CSC)
                nc.scalar.activation(sq[:, ds(SB, SB)], pb[:], AF.Square,
                                     bias=0.0, scale=1.0 / CSC)
                # all-ones stationary: every psum partition gets the full
                # sumsq, so no partition_broadcast is needed downstream
                smr = pp.tile([P, SB], F32, name=f"sm_{blk}_{ent}",
                              tag="sd", bufs=1)
                nc.tensor.matmul(smr[:], o128_sb[:],
                                 sq[:].rearrange("p (t s) -> p t s", t=2),
                                 start=True, stop=True, perf_mode=DRM)
                bsums[(blk, ent)] = smr
                # one psum->sbuf bf16 copy per half: rotation ops then run
                # in DVE 2x mode and the psum bank frees early
                pa16 = sp.tile([P, SB], BF16, name=f"pa16_{blk}_{ent}",
                               tag="p16", bufs=4)
                pb16 = sp.tile([P, SB], BF16, name=f"pb16_{blk}_{ent}",
                               tag="p16", bufs=4)
                nc.scalar.activation(pa16[:], pa[:], AF.Copy)
                nc.vector.tensor_copy(pb16[:], pb[:])
                bp16[(blk, ent)] = (pa16, pb16)

        def row_finish(blk, ent):
            """rr sqrt + rope + rmsnorm divide + fp8 hi/lo for one row."""
            sqscale, sqbias = rowconst[ent]
            thi, tlo = rowtiles[ent]
            tgt = xt_tiles[blk][2]
            tcos, tsin = tgt[:, 0:SB], tgt[:, SB : 2 * SB]
            smr = bsums.pop((blk, ent))
            rr = sp.tile([P, SB], BF16, name=f"rr_{blk}_{ent}", tag="lt", bufs=3)
            for hf in range(2):
                c = ds(hf * ATT_QB, ATT_QB)
                nc.scalar.activation(rr[:, c], smr[:, c],
                                     AF.Sqrt, bias=sqbias, scale=sqscale)
            pa16, pb16 = bp16.pop((blk, ent))
            nm = f"{blk}_{ent}"
            ta = sp.tile([P, SB], BF16, name=f"ta_{nm}", tag="rt", bufs=10)
            tb = sp.tile([P, SB], BF16, name=f"tb_{nm}", tag="rt", bufs=10)
            tc_ = sp.tile([P, SB], BF16, name=f"tc_{nm}", tag="rt", bufs=10)
            nc.vector.tensor_mul(ta[:], pa16[:], tcos)
            nc.vector.tensor_mul(tb[:], pb16[:], tsin)
            nc.vector.tensor_sub(tc_[:], ta[:], tb[:])
            td = sp.tile([P, SB], BF16, name=f"td_{nm}", tag="rt", bufs=10)
            te = sp.tile([P, SB], BF16, name=f"te_{nm}", tag="rt", bufs=10)
            tf = sp.tile([P, SB], BF16, name=f"tf_{nm}", tag="rt", bufs=10)
            nc.vector.tensor_mul(td[:], pb16[:], tcos)
            nc.vector.tensor_mul(te[:], pa16[:], tsin)
            nc.vector.tensor_add(tf[:], td[:], te[:])
            for hf in range(2):
                c = ds(hf * ATT_QB, ATT_QB)
                ca = ds(blk * SB + hf * ATT_QB, ATT_QB)
                qb = sp.tile([P, ATT_QB], BF16, name=f"qb_{nm}_{hf}",
                             tag="qb", bufs=8)
                with nc.allow_low_precision(reason="norm factor, 0.4% ok"):
                    nc.vector.reciprocal(qb[:], rr[:, c])
                for half, src in ((0, tc_), (1, tf)):
                    t0 = sp.tile([P, ATT_QB], BF16, name=f"t0_{nm}_{hf}_{half}",
                                 tag="t0", bufs=10)
                    nc.vector.tensor_mul(t0[:], src[:, c], qb[:])
                    hv = thi[:, ds(half, 1), ca].rearrange("p a q -> p (a q)")
                    lv = tlo[:, ds(half, 1), ca].rearrange("p a q -> p (a q)")
                    # k row gates attention start: keep its chain on the
                    # fast DVE; q rows split across Pool
                    if ent == "k":
                        nc.vector.tensor_copy(hv, t0[:])
                        nc.vector.tensor_sub(lv, t0[:], hv)
                    else:
                        nc.gpsimd.tensor_copy(hv, t0[:])
                        nc.gpsimd.tensor_sub(lv, t0[:], hv)

        def emit_v(blk):
            # v-proj: two seq-chunks share one psum bank; fp8 hi (Act copy
            # with x32 fold) + lo (DVE scalar_tensor_tensor)
            xth_t, xtl_t, _ = xt_tiles[blk]
            for sc2 in range(SB // P // 2):
                pv = pp.tile([P, 2 * DH], F32, name=f"pv_{blk}_{sc2}", tag="mm", bufs=4)
                for si, sc in enumerate((2 * sc2, 2 * sc2 + 1)):
                    i = 0
                    for xt_, wv_ in ((xth_t, wvh_sb), (xth_t, wvl_sb), (xtl_t, wvh_sb)):
                        for d8 in range(8):
                            nc.tensor.matmul(
                                pv[:, ds(si * DH, DH)],
                                xt_[:, ds(2 * d8, 2), ds(sc * P, P)],
                                wv_[:, ds(2 * d8, 2), :],
                                start=(si == 0 and i == 0),
                                stop=(si == 1 and i == 23), perf_mode=DRM)
                            i += 1
                nc.scalar.activation(
                    v_sb[:, ds(blk * 4 + 2 * sc2, 2), :].rearrange("p c d -> p (c d)"),
                    pv[:], AF.Copy, scale=1.0 / CSC)

        # ======== block 0 phase 1 (not pipelined) ========
        # k and q0 rows interleave pass-by-pass: the hh passes of both rows
        # need only wkh + wq-head0 + xth, so the PE streams right behind
        # the startup DMA instead of waiting for each row's full data
        issue_xt(1)
        for ti in range(3):
            proj_pass(0, "k", ti)
            proj_pass(0, "q0", ti)
        for ti in range(3):
            proj_pass(0, "q1", ti)
        for ent in ("k", "q0", "q1"):
            row_finish(0, ent)
        emit_v(0)

        for blk in range(NSB):
            nxt = blk + 1 if blk + 1 < NSB else None
            # ======== phase 2: attention, with PE fillers (deferred o-proj
            # of blk-1 sub1 + the k projection row of blk+1) interleaved so
            # the PE keeps running while DVE/Act/Pool chew the exp->mask->
            # hi/lo elementwise chains ========
            last_blk = nxt is None
            # k-row passes first: their Act-side tails (squares, psum
            # copies) land between early exps instead of after them all
            fillers = []
            if nxt is not None:
                fillers += [lambda t=t: proj_pass(nxt, "k", t) for t in range(3)]
            fillers += [lambda a=a: emit_oproj(*a) for a in pending_oproj]
            pending_oproj = []
            if blk == 0:
                # block 0 has no deferred o-proj and its attention groups
                # are thin (1-2 pairs): borrow the next block's q0 row
                fillers += [lambda t=t: proj_pass(1, "q0", t) for t in range(3)]
            fit = iter(fillers)

            def fill(n):
                for _ in range(n):
                    f = next(fit, None)
                    if f is not None:
                        f()

            if blk == 0:
                nc.sync.dma_start(msk_sb[:], masks.rearrange("m p j -> p m j"))
            # scores run TWO groups ahead of A@V/dn: the exp->mask->hi->lo
            # elementwise chain gets ~2 groups of PE work as slack
            st0 = att_scores(blk, 0, 0, fill=fill)
            fill(1)
            st1 = att_scores(blk, 0, 1)
            fill(1)
            st2 = att_scores(blk, 1, 0)
            att_avdn(st0)
            fill(1)
            st3 = att_scores(blk, 1, 1)
            att_avdn(st1)
            fill(1)
            if nxt is not None:
                row_finish(nxt, "k")
            else:
                # last block: sub-0 o-proj groups fill the final avdn stalls
                # (they need only the sub-0 avdns, both already emitted)
                emit_oproj(blk, 0, 0)
                emit_oproj(blk, 0, 1)
            att_avdn(st2)
            fill(2)
            if last_blk:
                emit_oproj(blk, 0, 2)
            att_avdn(st3)
            fill(4)

            # ======== phase 3: o-proj sub0 for blk (sub1 deferred), with
            # the next block's q-row passes interleaved as PE filler ========
            if blk == 0:
                nc.sync.dma_start(woh_sb[:], woh_d[:])
                nc.sync.dma_start(wol_sb[:], wol_d[:])
            qfill = []
            if nxt is not None:
                ents = ("q1",) if blk == 0 else ("q0", "q1")
                qfill = [(ent, t) for ent in ents for t in range(3)]
            qit = iter(qfill)
            for g4 in range(3 if last_blk else 0, 4):
                emit_oproj(blk, 0, g4)
                nq = next(qit, None)
                if nq is not None:
                    proj_pass(nxt, *nq)
            for sub in range(SB // ATT_QB):
                for g4 in range(4):
                    if sub == 0:
                        continue
                    if not last_blk:
                        pending_oproj.append((blk, sub, g4))
                    else:
                        emit_oproj(blk, sub, g4, split_dma=(g4 == 3))
            for nq in qit:
                proj_pass(nxt, *nq)

            # ======== phase 1 remainder for blk+1 (v first: its psum
            # copies drain before the next block's score burst) ========
            if nxt is not None:
                if nxt + 1 < NSB:
                    issue_xt(nxt + 1)
                emit_v(nxt)
                for ent in ("q0", "q1"):
                    row_finish(nxt, ent)

    nc.compile()
    return nc


_NC = None
LAST_RESULT = None


def _get_nc():
    global _NC
    if _NC is None:
        _NC = _build()
    return _NC


def _split8(x, scale):
    xs = np.asarray(x, np.float64) * scale
    hi = np.clip(xs, -240.0, 240.0).astype(NPFP8)
    lo = np.clip(xs - hi.astype(np.float64), -240.0, 240.0).astype(NPFP8)
    return hi, lo


def _host_tables(q_norm_w, k_norm_w):
    qw, kw = np.asarray(q_norm_w, np.float64), np.asarray(k_norm_w, np.float64)
    # device shares one cos/sin table across q/k and both rotary halves;
    # requires uniform (1 + w) factors (true for Gemma-zero-init norm weights)
    assert np.allclose(qw, qw[0]) and np.allclose(kw, kw[0]) and np.allclose(qw[0], kw[0]), \
        "non-uniform q/k norm weights need the 8-row trig layout"
    c = 1.0 + qw[0]
    inv_freq = 1.0 / (ROPE_BASE ** (np.arange(0, DH, 2, dtype=np.float64) / DH))
    freqs = np.outer(np.arange(S, dtype=np.float64), inv_freq)   # [S, DH/2]
    cos = (np.cos(freqs) * c).T.astype(NPBF16)                   # [DH/2, S]
    sin = (np.sin(freqs) * c).T.astype(NPBF16)
    trig = np.stack([cos, sin])                                  # [2, 128, S]

    i = np.arange(P)[:, None]
    j = np.arange(2 * ATT_QB)[None, :]
    # combined pair masks: row 0 causal (chunk d=0 | d=128), row 1 window
    # (chunk d=-SW | d=-SW+128); each 256-col half masks one chunk.
    cm0 = np.where(j < ATT_QB, j >= i, j - ATT_QB >= i + P)
    cm1 = np.where(j < ATT_QB, j <= i - 1, j - ATT_QB <= i + P - 1)
    masks = np.stack([cm0, cm1]).astype(NPBF16)
    o128 = np.ones((P, 2, P), NPFP8)
    o32 = np.full((P, 2, P), 1.0 / 32.0, NPFP8)
    return trig, masks, o128, o32


def _x_arrays(hidden_b):
    """hidden[b] [S, D] -> (hi, lo) arrays of shape [P, NSB, NDC, SB]."""
    xT = np.asarray(hidden_b, np.float64).T          # [D, S]
    hi, lo = _split8(xT, SW_X)
    def arr(a):
        return np.ascontiguousarray(
            a.reshape(NDC, P, NSB, SB).transpose(1, 2, 0, 3))
    return arr(hi), arr(lo)


def _w_arrays(Wq, Wk, Wv, Wo, g):
    """per-core weight slices -> prearranged fp8 hi/lo arrays."""
    res = {}
    for nm, w, nout in (("wq", Wq[g * EQ:(g + 1) * EQ], EQ),
                        ("wk", Wk[g * DH:(g + 1) * DH], DH),
                        ("wv", Wv[g * DH:(g + 1) * DH], DH)):
        hi, lo = _split8(np.asarray(w, np.float64).T, SW_W)   # [D, nout]
        for sfx, a in (("h", hi), ("l", lo)):
            a = np.ascontiguousarray(a.reshape(NDC, P, nout).transpose(1, 0, 2))
            if nm == "wq":
                # head-major: [P, 2, NDC*DH]
                a = np.ascontiguousarray(np.stack(
                    [a[:, :, hd * DH:(hd + 1) * DH].reshape(P, NDC * DH)
                     for hd in range(2)], axis=1))
            else:
                a = a.reshape(P, NDC * nout)   # flat dram layout (k, v)
            res[nm + sfx] = a
    hi, lo = _split8(np.asarray(Wo[:, g * EQ:(g + 1) * EQ], np.float64).T, SW_W)  # [EQ, D]
    for sfx, a in (("h", hi), ("l", lo)):
        res["wo" + sfx] = np.ascontiguousarray(
            a.reshape(4, P, D).transpose(1, 0, 2))
    return res


def _core_inputs(inputs, b, g, tables=None, xcache={}):
    if tables is None:
        tables = _host_tables(inputs["q_norm_w"], inputs["k_norm_w"])
    trig, masks, o128, o32 = tables
    key = (id(inputs), b)
    if key not in xcache:
        xcache.clear()
        for bb in range(B):
            xcache[(id(inputs), bb)] = _x_arrays(np.asarray(inputs["hidden_states"])[bb])
    xth, xtl = xcache[key]
    w = _w_arrays(np.asarray(inputs["Wq"]), np.asarray(inputs["Wk"]),
                  np.asarray(inputs["Wv"]), np.asarray(inputs["Wo"]), g)
    return {
        "xth": xth, "xtl": xtl,
        "wqh": w["wqh"], "wql": w["wql"],
        "wkh": w["wkh"], "wkl": w["wkl"],
        "wvh": w["wvh"], "wvl": w["wvl"],
        "woh": w["woh"], "wol": w["wol"],
        "trig": trig, "masks": masks, "o128f8": o128, "o32f8": o32,
        "cbias": np.tile(np.array([0.0, C2 * EPS, C2 * EPS / 256.0, NEGLNP],
                                  np.float32), (P, 1)),
    }


def kernel(hidden_states, Wq, Wk, Wv, Wo, q_norm_w, k_norm_w):
    global LAST_RESULT
    nc = _get_nc()
    inputs = {"hidden_states": hidden_states, "Wq": Wq, "Wk": Wk, "Wv": Wv,
              "Wo": Wo, "q_norm_w": q_norm_w, "k_norm_w": k_norm_w}
    tables = _host_tables(q_norm_w, k_norm_w)
    in_maps = [_core_inputs(inputs, core // 4, core % 4, tables)
               for core in range(8)]

    LAST_RESULT = run_bass_kernel_spmd(nc, in_maps, list(range(8)))
    res = LAST_RESULT.results
    outs = []
    for b in range(B):
        acc = np.zeros((D, S), np.float32)
        for g in range(4):
            acc += res[4 * b + g]["out"].astype(np.float32)
        outs.append(acc.T)
    return np.stack(outs).astype(np.float32)
